# revision 1
# baseline (speedup 1.0000x reference)
"""MoE (top-2 of 8 experts + shared expert) Trainium2 kernel, 8 NeuronCores.

Strategy
--------
Host (numpy): router matmul + top-2 + softmax gates (0.01% of FLOPs), token
dispatch (gather by expert), final combine (concat shared slices, scatter-add
gated expert outputs).

Device (8 cores, SPMD): core c computes
  1. expert c's FFN over the tokens routed to it (padded to capacity C):
     h = x @ w13.T ; a = silu(h[:, :F]) * h[:, F:] ; y = a @ w2.T
     The per-token gate g is folded into the *up* projection input on the
     host (g*x) so no broadcast multiply is needed on device:
     g*y == (silu(x@Wg.T) * ((g*x)@Wu.T)) @ w2.T.
  2. the shared-expert FFN for token slice [c*512, (c+1)*512).

All matmuls run as float32r (TF32: fp32 range, 10-bit mantissa, fp32
accumulation in PSUM) at full PE rate. Inputs are pre-rounded to TF32 on the
host (round-to-nearest-even), so no on-device casts are needed.

Everything is kept feature-major ("transposed": [feature, token]) so the
contraction dim is always the SBUF partition dim.
"""

import math

import numpy as np

import concourse.bass as bass
import concourse.mybir as mybir
import concourse.tile as tile
from concourse.bass_utils import run_bass_kernel_spmd

T, D, E, F, FS, TOP_K = 4096, 2048, 8, 4096, 4096, 2
NCORES = 8
P = 128
TS = T // NCORES  # shared-expert tokens per core
DK = D // P

F32 = mybir.dt.float32
MMDT = mybir.dt.float32r


def _split_multiwaits(nc):
    """This toolchain's walrus allows at most ONE fused sem-wait per
    instruction, but TileContext's assign_waits can emit several. Split the
    extras into standalone InstEventSemaphore instructions inserted
    immediately before the owning instruction on the same engine."""
    for fn in nc.m.functions:
        for bb in fn.blocks:
            insts = list(bb.instructions)
            out = []
            changed = False
            for inst in insts:
                si = inst.sync_info
                waits = list(si.on_wait) if (si and si.on_wait) else []
                if len(waits) > 1:
                    for w in waits[:-1]:
                        out.append(
                            mybir.InstEventSemaphore(
                                name=nc.get_next_instruction_name(),
                                engine=inst.engine,
                                ins=[],
                                outs=[],
                                sync_info=mybir.SyncInfo(on_wait=[w], on_update=[]),
                            )
                        )
                    inst.sync_info = mybir.SyncInfo(
                        on_wait=[waits[-1]], on_update=list(si.on_update)
                    )
                    changed = True
                out.append(inst)
            if changed:
                bb.instructions = out


def round_tf32(x: np.ndarray) -> np.ndarray:
    """Round fp32 to TF32 (10-bit mantissa), round-to-nearest-even."""
    u = np.ascontiguousarray(x, dtype=np.float32).view(np.uint32).copy()
    low = u & np.uint32(0x1FFF)
    bit13 = (u >> np.uint32(13)) & np.uint32(1)
    round_up = (low > 0x1000) | ((low == 0x1000) & (bit13 == 1))
    u = (u & ~np.uint32(0x1FFF)) + (round_up.astype(np.uint32) << np.uint32(13))
    return u.view(np.float32)


def _emit_ffn(nc, pools, xg_d, xu_d, w13_d, w2_d, out_d, n_tok, ct, fdim):
    """Emit one SwiGLU FFN: out[D, n_tok] = swiglu(x, w13) @ w2, transposed
    layouts everywhere. xg_d/xu_d: [DK, P, n_tok] gate/up inputs (may be the
    same tensor). w13_d: [DK, P, 2*fdim]. w2_d: [fdim//P, P, D].
    out_d: [DK, P, n_tok] (fp32)."""
    xp, wp, w2p, atp, op, ps1, ps2 = pools
    FT = fdim // P
    n_ch = n_tok // ct
    nspl = 1 if ct <= 512 else 2
    spl = ct // nspl
    G = 4 // nspl  # d-tiles per GEMM2 psum group

    silu = mybir.ActivationFunctionType.Silu
    xg_ap = xg_d[:].rearrange("k p c -> p k c")
    xu_ap = xu_d[:].rearrange("k p c -> p k c")
    w13_ap = w13_d[:].rearrange("k p f -> p k f")
    out_ap = out_d[:].rearrange("k p c -> p k c")

    for ch in range(n_ch):
        c0 = ch * ct
        xg = xp.tile([P, DK, ct], MMDT, tag="x", name="xg")
        nc.sync.dma_start(out=xg, in_=xg_ap[:, :, c0 : c0 + ct])
        aT = atp.tile([P, FT, ct], MMDT, tag="aT", name="aT")

        # gate half: aT = silu(x @ w13[:fdim].T)
        WFB = 2  # f-tiles per w13 DMA block (>=1KB per-partition lines)
        for ftp in range(FT // WFB):
            wt = wp.tile([P, DK, WFB * P], MMDT, tag="w13", name="wt")
            f0 = ftp * WFB * P
            nc.sync.dma_start(out=wt, in_=w13_ap[:, :, f0 : f0 + WFB * P])
            for fi in range(WFB):
                ft = ftp * WFB + fi
                for s in range(nspl):
                    pt = ps1.tile([P, 512], F32, tag="ps", name="pt")
                    for k in range(DK):
                        nc.tensor.matmul(
                            pt[:, :spl],
                            wt[:, k, fi * P : (fi + 1) * P],
                            xg[:, k, s * spl : (s + 1) * spl],
                            start=(k == 0),
                            stop=(k == DK - 1),
                        )
                    nc.scalar.activation(
                        out=aT[:, ft, s * spl : (s + 1) * spl],
                        in_=pt[:, :spl],
                        func=silu,
                    )

        # up half: aT *= (g*x) @ w13[fdim:].T
        if xu_d is xg_d:
            xu = xg
        else:
            xu = xp.tile([P, DK, ct], MMDT, tag="x", name="xu")
            nc.sync.dma_start(out=xu, in_=xu_ap[:, :, c0 : c0 + ct])
        for ftp in range(FT // WFB):
            wt = wp.tile([P, DK, WFB * P], MMDT, tag="w13", name="wt")
            f0 = (FT + ftp * WFB) * P
            nc.sync.dma_start(out=wt, in_=w13_ap[:, :, f0 : f0 + WFB * P])
            for fi in range(WFB):
                ft = ftp * WFB + fi
                for s in range(nspl):
                    pt = ps1.tile([P, 512], F32, tag="ps", name="pt")
                    for k in range(DK):
                        nc.tensor.matmul(
                            pt[:, :spl],
                            wt[:, k, fi * P : (fi + 1) * P],
                            xu[:, k, s * spl : (s + 1) * spl],
                            start=(k == 0),
                            stop=(k == DK - 1),
                        )
                    sl = aT[:, ft, s * spl : (s + 1) * spl]
                    nc.vector.tensor_mul(out=sl, in0=sl, in1=pt[:, :spl])

        # GEMM2: y[D, ct] = w2.T-contract over fdim, accumulated in PSUM
        for dg in range(DK // G):
            psy = [ps2.tile([P, 512], F32, tag="ps", name=f"psy{_i}") for _i in range(G * nspl)]
            for k in range(FT):
                w2t = w2p.tile([P, G * P], MMDT, tag="w2", name="w2t")
                nc.sync.dma_start(
                    out=w2t, in_=w2_d[:][k, :, dg * G * P : (dg + 1) * G * P]
                )
                for gi in range(G):
                    for s in range(nspl):
                        nc.tensor.matmul(
                            psy[gi * nspl + s][:, :spl],
                            w2t[:, gi * P : (gi + 1) * P],
                            aT[:, k, s * spl : (s + 1) * spl],
                            start=(k == 0),
                            stop=(k == FT - 1),
                        )
            for gi in range(G):
                ot = op.tile([P, ct], F32, tag="o", name="ot")
                for s in range(nspl):
                    nc.vector.tensor_copy(
                        out=ot[:, s * spl : (s + 1) * spl],
                        in_=psy[gi * nspl + s][:, :spl],
                    )
                nc.sync.dma_start(
                    out=out_ap[:, dg * G + gi, c0 : c0 + ct], in_=ot
                )


def build_program(C, CT):
    nc = bass.Bass()
    xeT = nc.dram_tensor("xeT", [DK, P, C], MMDT, kind="ExternalInput")
    xegT = nc.dram_tensor("xegT", [DK, P, C], MMDT, kind="ExternalInput")
    w13T = nc.dram_tensor("w13T", [DK, P, 2 * F], MMDT, kind="ExternalInput")
    w2T = nc.dram_tensor("w2T", [F // P, P, D], MMDT, kind="ExternalInput")
    xsT = nc.dram_tensor("xsT", [DK, P, TS], MMDT, kind="ExternalInput")
    sw13T = nc.dram_tensor("sw13T", [DK, P, 2 * FS], MMDT, kind="ExternalInput")
    sw2T = nc.dram_tensor("sw2T", [FS // P, P, D], MMDT, kind="ExternalInput")
    yeT = nc.dram_tensor("yeT", [DK, P, C], F32, kind="ExternalOutput")
    ysT = nc.dram_tensor("ysT", [DK, P, TS], F32, kind="ExternalOutput")

    with tile.TileContext(nc) as tc:
        with (
            tc.tile_pool(name="xp", bufs=2) as xp,
            tc.tile_pool(name="wp", bufs=4) as wp,
            tc.tile_pool(name="w2p", bufs=4) as w2p,
            tc.tile_pool(name="atp", bufs=1) as atp,
            tc.tile_pool(name="op", bufs=3) as op,
            tc.tile_pool(name="ps", bufs=8, space="PSUM") as ps,
        ):
            pools = (xp, wp, w2p, atp, op, ps, ps)
            _emit_ffn(nc, pools, xeT, xegT, w13T, w2T, yeT, C, CT, F)
            _emit_ffn(nc, pools, xsT, xsT, sw13T, sw2T, ysT, TS, TS, FS)
    _split_multiwaits(nc)
    return nc


_PROG_CACHE = {}

# test harnesses may override, e.g. {"trace": True, "trace_cores": [...]}
RUN_KWARGS = {}


def _get_program(C, CT):
    key = (C, CT)
    if key not in _PROG_CACHE:
        _PROG_CACHE[key] = build_program(C, CT)
    return _PROG_CACHE[key]


def kernel(x, router_DE, w13, w2, shared_w13, shared_w2):
    x = np.asarray(x, dtype=np.float32)
    router_DE = np.asarray(router_DE, dtype=np.float32)
    w13 = np.asarray(w13, dtype=np.float32)
    w2 = np.asarray(w2, dtype=np.float32)
    shared_w13 = np.asarray(shared_w13, dtype=np.float32)
    shared_w2 = np.asarray(shared_w2, dtype=np.float32)

    # ---- routing (host) ----
    logits = x @ router_DE  # [T, E]
    top_idx = np.argsort(-logits, axis=1, kind="stable")[:, :TOP_K]  # [T, K]
    top_vals = np.take_along_axis(logits, top_idx, axis=1)
    ex = np.exp(top_vals - top_vals.max(axis=1, keepdims=True))
    gates = (ex / ex.sum(axis=1, keepdims=True)).astype(np.float32)

    toks_per_e, gates_per_e = [], []
    for e in range(E):
        hit = top_idx == e  # [T, K]
        tok_mask = hit.any(axis=1)
        toks = np.nonzero(tok_mask)[0]
        g = (gates * hit).sum(axis=1)[toks].astype(np.float32)
        toks_per_e.append(toks)
        gates_per_e.append(g)

    max_cnt = max(len(t) for t in toks_per_e)
    # pick chunk count/size minimizing PE cycles: k chunks of CT tokens,
    # fp32r matmul cost ~ (N + 111) cycles, full rate needs 256 <= N <= 512
    best = None
    for k in range(1, 12):
        ct = math.ceil(max_cnt / k / 8) * 8
        if ct > 512:
            continue
        ct = max(ct, 256)
        cost = k * (ct + 111)
        if best is None or cost < best[0]:
            best = (cost, k, ct)
    _, k, CT = best
    C = k * CT

    # ---- host-side shard prep ----
    xT = np.ascontiguousarray(x.T)  # [D, T]
    xT_r = round_tf32(xT)
    sw13T = round_tf32(np.ascontiguousarray(shared_w13.T)).reshape(DK, P, 2 * FS)
    sw2T = round_tf32(np.ascontiguousarray(shared_w2.T)).reshape(FS // P, P, D)

    in_maps = []
    for c in range(NCORES):
        toks, g = toks_per_e[c], gates_per_e[c]
        cnt = len(toks)
        xe = np.zeros((D, C), np.float32)
        xe[:, :cnt] = xT_r[:, toks]
        xeg = np.zeros((D, C), np.float32)
        xeg[:, :cnt] = round_tf32(xT[:, toks] * g[None, :])
        in_maps.append(
            {
                "xeT": xe.reshape(DK, P, C),
                "xegT": xeg.reshape(DK, P, C),
                "w13T": round_tf32(np.ascontiguousarray(w13[c].T)).reshape(
                    DK, P, 2 * F
                ),
                "w2T": round_tf32(np.ascontiguousarray(w2[c].T)).reshape(
                    F // P, P, D
                ),
                "xsT": np.ascontiguousarray(xT_r[:, c * TS : (c + 1) * TS]).reshape(
                    DK, P, TS
                ),
                "sw13T": sw13T,
                "sw2T": sw2T,
            }
        )

    nc = _get_program(C, CT)
    res = run_bass_kernel_spmd(nc, in_maps, list(range(NCORES)), **RUN_KWARGS)
    kernel.last_result = res

    # ---- combine (host) ----
    out = np.empty((T, D), np.float32)
    for c in range(NCORES):
        out[c * TS : (c + 1) * TS] = res.results[c]["ysT"].reshape(D, TS).T
    for c in range(NCORES):
        toks = toks_per_e[c]
        ye = res.results[c]["yeT"].reshape(D, C)
        out[toks] += ye[:, : len(toks)].T
    return out



# revision 2
# speedup vs baseline: 1.8940x; 1.8940x over previous
"""MoE (top-2 of 8 experts + shared expert) Trainium2 kernel, 8 NeuronCores.

Strategy (v2)
-------------
Host (numpy): router matmul + top-2 + softmax gates, token dispatch (gather by
expert), weight pre-packing into PE-tile-major layouts, final combine
(scatter-add gated expert outputs + shared slices). Gates are applied on the
host at combine time, so the device computes the UNGATED expert FFN.

Device (8 cores, SPMD): core c computes
  1. expert c's FFN over the tokens routed to it (padded to capacity C)
  2. the shared-expert FFN for token slice [c*512, (c+1)*512)

All matmuls in bf16 (fp32 PSUM accumulation). bf16 runs at the same PE rate
as fp32r (1 row/cycle) but halves DMA traffic and SBUF footprint, and has no
small-N rate penalty.

Key change vs v1: weights stream from HBM exactly ONCE. x and the SwiGLU
activations aT stay resident in SBUF for the whole FFN; for each weight tile
we loop over all token chunks (v1 re-streamed all weights per 368-token
chunk, tripling DMA traffic and stalling the PE).

Loop structure per FFN (feature-major layouts, contraction on partitions):
  GEMM1: for each of 2*FT f-tiles: load w13 tile [P, DK, P] once;
         accumulate over DK k-steps into one PSUM bank per 512-token chunk;
         silu (gate half) / multiply-into-aT (up half).
  GEMM2: for each of DK d-tiles: load w2 tile [P, FT, P] once;
         accumulate over FT f-steps into one PSUM bank per chunk; copy to
         SBUF, DMA out.
Emission order routed-G1, shared-G1, routed-G2, shared-G2 hides the
aT-ready bubble between a FFN's GEMM1 and GEMM2.
"""

import math

import numpy as np
import ml_dtypes

import concourse.bass as bass
import concourse.mybir as mybir
import concourse.tile as tile
from concourse.bass_utils import run_bass_kernel_spmd

T, D, E, F, FS, TOP_K = 4096, 2048, 8, 4096, 4096, 2
NCORES = 8
P = 128
TS = T // NCORES  # shared-expert tokens per core
DK = D // P  # 16 k-tiles over D
FT = F // P  # 32 f-tiles over F

F32 = mybir.dt.float32
BF16 = mybir.dt.bfloat16
NP_BF16 = ml_dtypes.bfloat16


def _split_multiwaits(nc):
    """This toolchain's walrus allows at most ONE fused sem-wait per
    instruction, but TileContext's assign_waits can emit several. Split the
    extras into standalone InstEventSemaphore instructions inserted
    immediately before the owning instruction on the same engine."""
    for fn in nc.m.functions:
        for bb in fn.blocks:
            insts = list(bb.instructions)
            out = []
            changed = False
            for inst in insts:
                si = inst.sync_info
                waits = list(si.on_wait) if (si and si.on_wait) else []
                if len(waits) > 1:
                    for w in waits[:-1]:
                        out.append(
                            mybir.InstEventSemaphore(
                                name=nc.get_next_instruction_name(),
                                engine=inst.engine,
                                ins=[],
                                outs=[],
                                sync_info=mybir.SyncInfo(on_wait=[w], on_update=[]),
                            )
                        )
                    inst.sync_info = mybir.SyncInfo(
                        on_wait=[waits[-1]], on_update=list(si.on_update)
                    )
                    changed = True
                out.append(inst)
            if changed:
                bb.instructions = out


def _chunks(n):
    """512-token chunks covering n."""
    return [(i * 512, min(512, n - i * 512)) for i in range(math.ceil(n / 512))]


def _emit_gemm1(nc, pools, xt, at, w13_d, chunks, col0):
    """aT[:, ft, col0:col0+n] = silu(x@Wg.T) * (x@Wu.T), columns from xt."""
    wp, ps = pools
    silu = mybir.ActivationFunctionType.Silu
    for ft in range(2 * FT):
        wt = wp.tile([P, DK, P], BF16, tag="w13", name="wt")
        nc.sync.dma_start(out=wt, in_=w13_d[:][:, ft])
        pts = []
        for s, (c0, cn) in enumerate(chunks):
            pts.append(ps.tile([P, 512], F32, tag="ps", name=f"p{s}"))
        for k in range(DK):
            for s, (c0, cn) in enumerate(chunks):
                nc.tensor.matmul(
                    pts[s][:, :cn],
                    wt[:, k],
                    xt[:, k, col0 + c0 : col0 + c0 + cn],
                    start=(k == 0),
                    stop=(k == DK - 1),
                )
        fi = ft if ft < FT else ft - FT
        for s, (c0, cn) in enumerate(chunks):
            sl = at[:, fi, col0 + c0 : col0 + c0 + cn]
            if ft < FT:
                nc.scalar.activation(out=sl, in_=pts[s][:, :cn], func=silu)
            else:
                nc.vector.tensor_mul(out=sl, in0=sl, in1=pts[s][:, :cn])


def _emit_gemm2(nc, pools, at, w2_d, out_d, chunks, col0, n_tok):
    """out[dt, :, :] = aT @ w2, columns [col0, col0+n_tok) of at."""
    w2p, op, ps = pools
    for dt in range(DK):
        w2t = w2p.tile([P, FT, P], BF16, tag="w2", name="w2t")
        nc.sync.dma_start(out=w2t, in_=w2_d[:][:, dt])
        pys = []
        for s, (c0, cn) in enumerate(chunks):
            pys.append(ps.tile([P, 512], F32, tag="ps", name=f"py{s}"))
        for kf in range(FT):
            for s, (c0, cn) in enumerate(chunks):
                nc.tensor.matmul(
                    pys[s][:, :cn],
                    w2t[:, kf],
                    at[:, kf, col0 + c0 : col0 + c0 + cn],
                    start=(kf == 0),
                    stop=(kf == FT - 1),
                )
        ot = op.tile([P, n_tok], F32, tag="o", name="ot")
        for s, (c0, cn) in enumerate(chunks):
            nc.vector.tensor_copy(out=ot[:, c0 : c0 + cn], in_=pys[s][:, :cn])
        nc.sync.dma_start(out=out_d[:][dt], in_=ot)


def build_program(C):
    nc = bass.Bass()
    xeT = nc.dram_tensor("xeT", [DK, P, C], BF16, kind="ExternalInput")
    xsT = nc.dram_tensor("xsT", [DK, P, TS], BF16, kind="ExternalInput")
    # w13 packed [p, ft, k, fo]: tile (ft) is [P, DK, P], stationary for GEMM1
    w13p = nc.dram_tensor("w13p", [P, 2 * FT, DK, P], BF16, kind="ExternalInput")
    # w2 packed [p, dt, kf, do]: tile (dt) is [P, FT, P], stationary for GEMM2
    w2p_d = nc.dram_tensor("w2p", [P, DK, FT, P], BF16, kind="ExternalInput")
    sw13p = nc.dram_tensor("sw13p", [P, 2 * FT, DK, P], BF16, kind="ExternalInput")
    sw2p_d = nc.dram_tensor("sw2p", [P, DK, FT, P], BF16, kind="ExternalInput")
    yeT = nc.dram_tensor("yeT", [DK, P, C], F32, kind="ExternalOutput")
    ysT = nc.dram_tensor("ysT", [DK, P, TS], F32, kind="ExternalOutput")

    ch_r = _chunks(C)
    ch_s = _chunks(TS)

    with tile.TileContext(nc) as tc:
        with (
            tc.tile_pool(name="xp", bufs=1) as xp,
            tc.tile_pool(name="ap", bufs=1) as ap,
            tc.tile_pool(name="wp", bufs=3) as wp,
            tc.tile_pool(name="w2p", bufs=2) as w2p,
            tc.tile_pool(name="op", bufs=2) as op,
            tc.tile_pool(name="ps", bufs=8, space="PSUM") as ps,
        ):
            # persistent tiles: x and aT for routed [0:C) + shared [C:C+TS)
            xt = xp.tile([P, DK, C + TS], BF16, tag="x", name="xt")
            nc.sync.dma_start(
                out=xt[:, :, :C], in_=xeT[:].rearrange("k p c -> p k c")
            )
            nc.sync.dma_start(
                out=xt[:, :, C:], in_=xsT[:].rearrange("k p c -> p k c")
            )
            at = ap.tile([P, FT, C + TS], BF16, tag="aT", name="at")

            _emit_gemm1(nc, (wp, ps), xt, at, w13p, ch_r, 0)
            _emit_gemm1(nc, (wp, ps), xt, at, sw13p, ch_s, C)
            _emit_gemm2(nc, (w2p, op, ps), at, w2p_d, yeT, ch_r, 0, C)
            _emit_gemm2(nc, (w2p, op, ps), at, sw2p_d, ysT, ch_s, C, TS)
    _split_multiwaits(nc)
    return nc


_PROG_CACHE = {}

# test harnesses may override, e.g. {"trace": True, "trace_cores": [...]}
RUN_KWARGS = {}


def _get_program(C):
    if C not in _PROG_CACHE:
        _PROG_CACHE[C] = build_program(C)
    return _PROG_CACHE[C]


def _pack_w13(w):
    """[2F, D] fp32 -> [p, ft, k, fo] bf16 (PE stationary tiles)."""
    return np.ascontiguousarray(
        w.astype(NP_BF16).reshape(2 * FT, P, DK, P).transpose(3, 0, 2, 1)
    )


def _pack_w2(w):
    """[D, F] fp32 -> [p, dt, kf, do] bf16 (PE stationary tiles)."""
    return np.ascontiguousarray(
        w.astype(NP_BF16).reshape(DK, P, FT, P).transpose(3, 0, 2, 1)
    )


def kernel(x, router_DE, w13, w2, shared_w13, shared_w2):
    x = np.asarray(x, dtype=np.float32)
    router_DE = np.asarray(router_DE, dtype=np.float32)
    w13 = np.asarray(w13, dtype=np.float32)
    w2 = np.asarray(w2, dtype=np.float32)
    shared_w13 = np.asarray(shared_w13, dtype=np.float32)
    shared_w2 = np.asarray(shared_w2, dtype=np.float32)

    # ---- routing (host) ----
    logits = x @ router_DE  # [T, E]
    top_idx = np.argsort(-logits, axis=1, kind="stable")[:, :TOP_K]  # [T, K]
    top_vals = np.take_along_axis(logits, top_idx, axis=1)
    ex = np.exp(top_vals - top_vals.max(axis=1, keepdims=True))
    gates = (ex / ex.sum(axis=1, keepdims=True)).astype(np.float32)

    toks_per_e, gates_per_e = [], []
    for e in range(E):
        hit = top_idx == e  # [T, K]
        toks = np.nonzero(hit.any(axis=1))[0]
        g = (gates * hit).sum(axis=1)[toks].astype(np.float32)
        toks_per_e.append(toks)
        gates_per_e.append(g)

    max_cnt = max(len(t) for t in toks_per_e)
    C = math.ceil(max_cnt / 8) * 8

    # ---- host-side shard prep ----
    xT = np.ascontiguousarray(x.T).astype(NP_BF16)  # [D, T]
    sw13pk = _pack_w13(shared_w13)
    sw2pk = _pack_w2(shared_w2)

    in_maps = []
    for c in range(NCORES):
        toks = toks_per_e[c]
        xe = np.zeros((D, C), NP_BF16)
        xe[:, : len(toks)] = xT[:, toks]
        in_maps.append(
            {
                "xeT": xe.reshape(DK, P, C),
                "xsT": np.ascontiguousarray(
                    xT[:, c * TS : (c + 1) * TS]
                ).reshape(DK, P, TS),
                "w13p": _pack_w13(w13[c]),
                "w2p": _pack_w2(w2[c]),
                "sw13p": sw13pk,
                "sw2p": sw2pk,
            }
        )

    nc = _get_program(C)
    res = run_bass_kernel_spmd(nc, in_maps, list(range(NCORES)), **RUN_KWARGS)
    kernel.last_result = res

    # ---- combine (host) ----
    out = np.empty((T, D), np.float32)
    for c in range(NCORES):
        out[c * TS : (c + 1) * TS] = res.results[c]["ysT"].reshape(D, TS).T
    for c in range(NCORES):
        toks, g = toks_per_e[c], gates_per_e[c]
        ye = res.results[c]["yeT"].reshape(D, C)[:, : len(toks)]
        out[toks] += (ye * g[None, :]).T
    return out


# revision 5
# speedup vs baseline: 1.9310x; 1.0196x over previous
"""MoE (top-2 of 8 experts + shared expert) Trainium2 kernel, 8 NeuronCores.

Strategy (v2)
-------------
Host (numpy): router matmul + top-2 + softmax gates, token dispatch (gather by
expert), weight pre-packing into PE-tile-major layouts, final combine
(scatter-add gated expert outputs + shared slices). Gates are applied on the
host at combine time, so the device computes the UNGATED expert FFN.

Device (8 cores, SPMD): core c computes
  1. expert c's FFN over the tokens routed to it (padded to capacity C)
  2. the shared-expert FFN for token slice [c*512, (c+1)*512)

All matmuls in bf16 (fp32 PSUM accumulation). bf16 runs at the same PE rate
as fp32r (1 row/cycle) but halves DMA traffic and SBUF footprint, and has no
small-N rate penalty.

Key change vs v1: weights stream from HBM exactly ONCE. x and the SwiGLU
activations aT stay resident in SBUF for the whole FFN; for each weight tile
we loop over all token chunks (v1 re-streamed all weights per 368-token
chunk, tripling DMA traffic and stalling the PE).

Loop structure per FFN (feature-major layouts, contraction on partitions):
  GEMM1: for each of 2*FT f-tiles: load w13 tile [P, DK, P] once;
         accumulate over DK k-steps into one PSUM bank per 512-token chunk;
         silu (gate half) / multiply-into-aT (up half).
  GEMM2: for each of DK d-tiles: load w2 tile [P, FT, P] once;
         accumulate over FT f-steps into one PSUM bank per chunk; copy to
         SBUF, DMA out.
Emission order routed-G1, shared-G1, routed-G2, shared-G2 hides the
aT-ready bubble between a FFN's GEMM1 and GEMM2.
"""

import math

import numpy as np
import ml_dtypes

import concourse.bass as bass
import concourse.mybir as mybir
import concourse.tile as tile
from concourse.bass_utils import run_bass_kernel_spmd

T, D, E, F, FS, TOP_K = 4096, 2048, 8, 4096, 4096, 2
NCORES = 8
P = 128
TS = T // NCORES  # shared-expert tokens per core
DK = D // P  # 16 k-tiles over D
FT = F // P  # 32 f-tiles over F

F32 = mybir.dt.float32
BF16 = mybir.dt.bfloat16
NP_BF16 = ml_dtypes.bfloat16


def _split_multiwaits(nc):
    """This toolchain's walrus allows at most ONE fused sem-wait per
    instruction, but TileContext's assign_waits can emit several. Split the
    extras into standalone InstEventSemaphore instructions inserted
    immediately before the owning instruction on the same engine."""
    for fn in nc.m.functions:
        for bb in fn.blocks:
            insts = list(bb.instructions)
            out = []
            changed = False
            for inst in insts:
                si = inst.sync_info
                waits = list(si.on_wait) if (si and si.on_wait) else []
                if len(waits) > 1:
                    for w in waits[:-1]:
                        out.append(
                            mybir.InstEventSemaphore(
                                name=nc.get_next_instruction_name(),
                                engine=inst.engine,
                                ins=[],
                                outs=[],
                                sync_info=mybir.SyncInfo(on_wait=[w], on_update=[]),
                            )
                        )
                    inst.sync_info = mybir.SyncInfo(
                        on_wait=[waits[-1]], on_update=list(si.on_update)
                    )
                    changed = True
                out.append(inst)
            if changed:
                bb.instructions = out


def _chunks(n):
    """512-token chunks covering n."""
    return [(i * 512, min(512, n - i * 512)) for i in range(math.ceil(n / 512))]


def _emit_gemm1(nc, pools, xt, at, w13_d, chunks, col0):
    """aT[:, ft, col0:col0+n] = silu(x@Wg.T) * (x@Wu.T), columns from xt."""
    wp, ps = pools
    silu = mybir.ActivationFunctionType.Silu
    for ft in range(2 * FT):
        wt = wp.tile([P, DK, P], BF16, tag="w13", name="wt")
        nc.sync.dma_start(out=wt, in_=w13_d[:][:, ft])
        pts = []
        for s, (c0, cn) in enumerate(chunks):
            pts.append(ps.tile([P, 512], F32, tag="ps", name=f"p{s}"))
        for k in range(DK):
            for s, (c0, cn) in enumerate(chunks):
                nc.tensor.matmul(
                    pts[s][:, :cn],
                    wt[:, k],
                    xt[:, k, col0 + c0 : col0 + c0 + cn],
                    start=(k == 0),
                    stop=(k == DK - 1),
                )
        fi = ft if ft < FT else ft - FT
        for s, (c0, cn) in enumerate(chunks):
            sl = at[:, fi, col0 + c0 : col0 + c0 + cn]
            if ft < FT:
                nc.scalar.activation(out=sl, in_=pts[s][:, :cn], func=silu)
            else:
                nc.vector.tensor_mul(out=sl, in0=sl, in1=pts[s][:, :cn])


def _emit_gemm2(nc, pools, at, w2_d, out_d, chunks, col0, n_tok):
    """out[dt, :, :] = aT @ w2, columns [col0, col0+n_tok) of at."""
    w2p, op, ps = pools
    for dt in range(DK):
        w2t = w2p.tile([P, FT, P], BF16, tag="w2", name="w2t")
        # scalar (Activation) HWDGE queue: keeps w2 prefetch off the sync
        # queue, which carries the output writes during GEMM2
        nc.scalar.dma_start(out=w2t, in_=w2_d[:][:, dt])
        pys = []
        for s, (c0, cn) in enumerate(chunks):
            pys.append(ps.tile([P, 512], F32, tag="ps", name=f"py{s}"))
        for kf in range(FT):
            for s, (c0, cn) in enumerate(chunks):
                nc.tensor.matmul(
                    pys[s][:, :cn],
                    w2t[:, kf],
                    at[:, kf, col0 + c0 : col0 + c0 + cn],
                    start=(kf == 0),
                    stop=(kf == FT - 1),
                )
        ot = op.tile([P, n_tok], F32, tag="o", name="ot")
        for s, (c0, cn) in enumerate(chunks):
            nc.vector.tensor_copy(out=ot[:, c0 : c0 + cn], in_=pys[s][:, :cn])
        nc.sync.dma_start(out=out_d[:][dt], in_=ot)


def build_program(C):
    nc = bass.Bass()
    xeT = nc.dram_tensor("xeT", [DK, P, C], BF16, kind="ExternalInput")
    xsT = nc.dram_tensor("xsT", [DK, P, TS], BF16, kind="ExternalInput")
    # w13 packed [p, ft, k, fo]: tile (ft) is [P, DK, P], stationary for GEMM1
    w13p = nc.dram_tensor("w13p", [P, 2 * FT, DK, P], BF16, kind="ExternalInput")
    # w2 packed [p, dt, kf, do]: tile (dt) is [P, FT, P], stationary for GEMM2
    w2p_d = nc.dram_tensor("w2p", [P, DK, FT, P], BF16, kind="ExternalInput")
    sw13p = nc.dram_tensor("sw13p", [P, 2 * FT, DK, P], BF16, kind="ExternalInput")
    sw2p_d = nc.dram_tensor("sw2p", [P, DK, FT, P], BF16, kind="ExternalInput")
    yeT = nc.dram_tensor("yeT", [DK, P, C], F32, kind="ExternalOutput")
    ysT = nc.dram_tensor("ysT", [DK, P, TS], F32, kind="ExternalOutput")

    ch_r = _chunks(C)
    ch_s = _chunks(TS)

    with tile.TileContext(nc) as tc:
        with (
            tc.tile_pool(name="xp", bufs=1) as xp,
            tc.tile_pool(name="ap", bufs=1) as ap,
            tc.tile_pool(name="wp", bufs=3) as wp,
            tc.tile_pool(name="w2p", bufs=3) as w2p,
            tc.tile_pool(name="op", bufs=2) as op,
            tc.tile_pool(name="ps", bufs=8, space="PSUM") as ps,
        ):
            # persistent tiles: x and aT for routed [0:C) + shared [C:C+TS)
            # x loads ride the scalar (Activation) HWDGE queue, split per
            # k-tile, so they overlap the w13 prefetch on the sync queue and
            # the first GEMM1 f-tile starts ~4us in instead of ~32us.
            xt = xp.tile([P, DK, C + TS], BF16, tag="x", name="xt")
            for k in range(DK):
                nc.scalar.dma_start(out=xt[:, k, :C], in_=xeT[:][k])
            for k in range(DK):
                nc.scalar.dma_start(out=xt[:, k, C:], in_=xsT[:][k])
            at = ap.tile([P, FT, C + TS], BF16, tag="aT", name="at")

            _emit_gemm1(nc, (wp, ps), xt, at, w13p, ch_r, 0)
            _emit_gemm1(nc, (wp, ps), xt, at, sw13p, ch_s, C)
            _emit_gemm2(nc, (w2p, op, ps), at, w2p_d, yeT, ch_r, 0, C)
            _emit_gemm2(nc, (w2p, op, ps), at, sw2p_d, ysT, ch_s, C, TS)
    _split_multiwaits(nc)
    return nc


_PROG_CACHE = {}

# test harnesses may override, e.g. {"trace": True, "trace_cores": [...]}
RUN_KWARGS = {}


def _get_program(C):
    if C not in _PROG_CACHE:
        _PROG_CACHE[C] = build_program(C)
    return _PROG_CACHE[C]


def _pack_w13(w):
    """[2F, D] fp32 -> [p, ft, k, fo] bf16 (PE stationary tiles)."""
    return np.ascontiguousarray(
        w.astype(NP_BF16).reshape(2 * FT, P, DK, P).transpose(3, 0, 2, 1)
    )


def _pack_w2(w):
    """[D, F] fp32 -> [p, dt, kf, do] bf16 (PE stationary tiles)."""
    return np.ascontiguousarray(
        w.astype(NP_BF16).reshape(DK, P, FT, P).transpose(3, 0, 2, 1)
    )


def kernel(x, router_DE, w13, w2, shared_w13, shared_w2):
    x = np.asarray(x, dtype=np.float32)
    router_DE = np.asarray(router_DE, dtype=np.float32)
    w13 = np.asarray(w13, dtype=np.float32)
    w2 = np.asarray(w2, dtype=np.float32)
    shared_w13 = np.asarray(shared_w13, dtype=np.float32)
    shared_w2 = np.asarray(shared_w2, dtype=np.float32)

    # ---- routing (host) ----
    logits = x @ router_DE  # [T, E]
    top_idx = np.argsort(-logits, axis=1, kind="stable")[:, :TOP_K]  # [T, K]
    top_vals = np.take_along_axis(logits, top_idx, axis=1)
    ex = np.exp(top_vals - top_vals.max(axis=1, keepdims=True))
    gates = (ex / ex.sum(axis=1, keepdims=True)).astype(np.float32)

    toks_per_e, gates_per_e = [], []
    for e in range(E):
        hit = top_idx == e  # [T, K]
        toks = np.nonzero(hit.any(axis=1))[0]
        g = (gates * hit).sum(axis=1)[toks].astype(np.float32)
        toks_per_e.append(toks)
        gates_per_e.append(g)

    max_cnt = max(len(t) for t in toks_per_e)
    C = math.ceil(max_cnt / 8) * 8

    # ---- host-side shard prep ----
    xT = np.ascontiguousarray(x.T).astype(NP_BF16)  # [D, T]
    sw13pk = _pack_w13(shared_w13)
    sw2pk = _pack_w2(shared_w2)

    in_maps = []
    for c in range(NCORES):
        toks = toks_per_e[c]
        xe = np.zeros((D, C), NP_BF16)
        xe[:, : len(toks)] = xT[:, toks]
        in_maps.append(
            {
                "xeT": xe.reshape(DK, P, C),
                "xsT": np.ascontiguousarray(
                    xT[:, c * TS : (c + 1) * TS]
                ).reshape(DK, P, TS),
                "w13p": _pack_w13(w13[c]),
                "w2p": _pack_w2(w2[c]),
                "sw13p": sw13pk,
                "sw2p": sw2pk,
            }
        )

    nc = _get_program(C)
    res = run_bass_kernel_spmd(nc, in_maps, list(range(NCORES)), **RUN_KWARGS)
    kernel.last_result = res

    # ---- combine (host) ----
    out = np.empty((T, D), np.float32)
    for c in range(NCORES):
        out[c * TS : (c + 1) * TS] = res.results[c]["ysT"].reshape(D, TS).T
    for c in range(NCORES):
        toks, g = toks_per_e[c], gates_per_e[c]
        ye = res.results[c]["yeT"].reshape(D, C)[:, : len(toks)]
        out[toks] += (ye * g[None, :]).T
    return out
